# revision 1
# baseline (speedup 1.0000x reference)
"""Multi-head self-attention on 8 Trainium2 NeuronCores.

Problem: x:(4,2048,1024) fp32; q = x@Wq, kv = x@Wkv (k,v split), 8 heads of
dim 64, softmax(q k^T / 8) v, concat heads, @Wo + bo -> (4,2048,1024).

Sharding: core c handles batch b=c//2 and head group g=c%2 (4 of 8 heads).
Each core computes its batch's projections restricted to its 4 heads, full
attention for those heads, and a partial output projection y_c = U_norm @ Wo_g.
Host gathers: out[b] = y_{2b} + y_{2b+1} + bo  (the "all-reduce" of the
tensor-parallel head split, done at unshard time).

Device algorithm (per core), all matmul operands fp16, PSUM accumulate fp32:
  - host supplies xT = x[b].T so the contraction dim (QDIM) is the partition
    axis; projections compute qT/kT (head_dim-major) and v (seq-major) tiles.
  - attention per head, per i-half (1024 q rows), per j-tile (128 k rows):
      simT[j,i] = kT_h(j)^T-tile @ qT_h        (PE, K=64)
      expT = exp(SCALE*simT)                   (ACT, reads PSUM directly)
      U~[d,i] += [v_h | 1]^T @ expT            (PE, K=128; row 64 = softmax sum)
    then normalization: r = 1/s via fast-reciprocal (DVE), R = ones x r
    broadcast (PE K=1 matmul), U_norm = U~ * R (DVE).
  - y[m,:] = U_norm_pairs^T @ Wo_g (K=128 per head pair), DVE drain, DMA out.
"""

import numpy as np

# ---- problem constants (hardcoded per the harness contract) ----
B, N, QDIM = 4, 2048, 1024
HEADS, DIM_MODEL = 8, 512
HEAD_DIM = DIM_MODEL // HEADS  # 64
SCALE = HEAD_DIM ** -0.5  # 0.125
N_CORES = 8
HEADS_PER_CORE = HEADS // 2  # 4 (head-group split across 2 cores per batch)
DMC = HEADS_PER_CORE * HEAD_DIM  # 256 per-core model dim slice


def build_nc(seq=N, qd=QDIM, nh=HEADS_PER_CORE, hd=HEAD_DIM, dout=QDIM,
             scale=SCALE, ihw=1024, skip_norm=False, norm_mode='dve',
             expp_bufs=3, upool_bufs=2, rows_bufs=2, ysb_bufs=3,
             phases='all', simp_bufs=2, uaccp_bufs=1, spare_bufs=2,
             xt_one_dma=True, y_pair_dma=True):
    """Build the per-core Bass program (same program on all 8 cores)."""
    from contextlib import ExitStack

    import concourse.bass as bass
    import concourse.tile as tile
    from concourse import bacc, mybir

    P = 128
    NC5 = 512  # psum bank width in fp32
    f16 = mybir.dt.float16
    f32 = mybir.dt.float32
    Exp = mybir.ActivationFunctionType.Exp
    Ln = mybir.ActivationFunctionType.Ln

    dmc = nh * hd                 # per-core projected dim (256)
    kt = qd // P                  # contraction tiles over QDIM (8)
    seqt = seq // P               # seq tiles (16)
    mtiles = max(1, dmc // P)     # qT/kT partition tiles (2)
    heads_per_mtile = nh // mtiles
    ihw = min(ihw, seq)           # i-half width
    n_ih = seq // ihw
    npairs = mtiles               # head pairs stacked for final proj (2)

    def chunks(total, w=NC5):
        c0 = 0
        while c0 < total:
            yield c0, min(w, total - c0)
            c0 += w

    nc = bacc.Bacc("TRN2", target_bir_lowering=False, debug=False,
                   num_devices=N_CORES)

    xt = nc.dram_tensor("xt", (qd, seq), f16, kind="ExternalInput").ap()
    wq = nc.dram_tensor("wq", (qd, dmc), f16, kind="ExternalInput").ap()
    wk = nc.dram_tensor("wk", (qd, dmc), f16, kind="ExternalInput").ap()
    wv = nc.dram_tensor("wv", (qd, dmc), f16, kind="ExternalInput").ap()
    wo = nc.dram_tensor("wo", (dmc, dout), f16, kind="ExternalInput").ap()
    y = nc.dram_tensor("y", (seq, dout), f32, kind="ExternalOutput").ap()

    with tile.TileContext(nc) as tc, ExitStack() as ctx:
        # ---- SBUF pools ----
        persist = ctx.enter_context(tc.tile_pool(name="persist", bufs=1))
        expp = ctx.enter_context(tc.tile_pool(name="expp", bufs=expp_bufs))
        upool = ctx.enter_context(tc.tile_pool(name="upool", bufs=upool_bufs))
        rows = ctx.enter_context(tc.tile_pool(name="rows", bufs=rows_bufs))
        ysb = ctx.enter_context(tc.tile_pool(name="ysb", bufs=ysb_bufs))
        # ---- PSUM pools (8 banks total: 2 spare + 4 sim + 2 uacc) ----
        spare = ctx.enter_context(tc.tile_pool(name="spare", bufs=spare_bufs, space="PSUM"))
        simp = ctx.enter_context(tc.tile_pool(name="simp", bufs=simp_bufs, space="PSUM"))
        uaccp = ctx.enter_context(tc.tile_pool(name="uaccp", bufs=uaccp_bufs, space="PSUM"))

        # ---- persistent SBUF tensors ----
        xt_sb = persist.tile([P, kt, seq], f16)
        wq_sb = persist.tile([P, kt, dmc], f16)
        wk_sb = persist.tile([P, kt, dmc], f16)
        wv_sb = persist.tile([P, kt, dmc], f16)
        wo_sb = persist.tile([min(P, dmc), npairs, dout], f16)
        v_sb = persist.tile([P, seqt, nh, hd + 1], f16)
        qt_sb = persist.tile([min(P, dmc), mtiles, seq], f16)
        kt_sb = persist.tile([min(P, dmc), mtiles, seq], f16)
        upairs = [persist.tile([min(P, dmc), seq], f16, name=f"upair{p}")
                  for p in range(npairs)]
        ones65 = persist.tile([65, hd], f16)

        # ---- input loads ----
        if xt_one_dma:
            nc.sync.dma_start(xt_sb[:], xt.rearrange("(ko ki) s -> ki ko s",
                                                     ki=P))
        else:
            for ko in range(kt):
                nc.sync.dma_start(xt_sb[:, ko, :], xt[ko * P:(ko + 1) * P, :])
        nc.sync.dma_start(wk_sb[:], wk.rearrange("(ko ki) m -> ki ko m", ki=P))
        nc.sync.dma_start(wq_sb[:], wq.rearrange("(ko ki) m -> ki ko m", ki=P))
        nc.sync.dma_start(wv_sb[:], wv.rearrange("(ko ki) m -> ki ko m", ki=P))
        nc.sync.dma_start(wo_sb[:], wo.rearrange("(t p) n -> p t n", p=min(P, dmc)))
        nc.vector.memset(v_sb[:, :, :, hd:hd + 1], 1.0)
        nc.vector.memset(ones65[:], 1.0)

        def proj_kq_tile(mt, which, n0, nw):
            """One [mp, nw] tile of kT (which=0) or qT (which=1) for m-tile mt."""
            mp = min(P, dmc)
            w_sb, out_sb = ((wk_sb, kt_sb), (wq_sb, qt_sb))[which]
            ps = spare.tile([mp, NC5], f32, tag="ps512", name="ps")
            for ko in range(kt):
                nc.tensor.matmul(
                    ps[:, 0:nw],
                    lhsT=w_sb[:, ko, mt * mp:(mt + 1) * mp],
                    rhs=xt_sb[:, ko, n0:n0 + nw],
                    start=(ko == 0), stop=(ko == kt - 1))
            nc.vector.tensor_copy(
                out_sb[0:mp, mt, n0:n0 + nw], ps[:, 0:nw])

        def proj_v_tile(jt):
            """v natural layout [seq, dmc] -> v_sb[:, jt, h, 0:hd]."""
            ps = spare.tile([P, dmc], f32, tag="ps512", name="ps")
            for ko in range(kt):
                nc.tensor.matmul(
                    ps[:],
                    lhsT=xt_sb[:, ko, jt * P:(jt + 1) * P],
                    rhs=wv_sb[:, ko, :],
                    start=(ko == 0), stop=(ko == kt - 1))
            nc.vector.tensor_copy(
                v_sb[:, jt, :, 0:hd],
                ps.rearrange("p (h d) -> p h d", h=nh))

        def attn_head_ih(h, ih, nm_override=None, pre_norm_cb=None):
            if True:
                nmode = nm_override or norm_mode
                mt = h // heads_per_mtile
                hb = (h % heads_per_mtile) * hd
                pair = h // heads_per_mtile
                i0 = ih * ihw
                uacc = uaccp.tile([hd + 1, ihw], f32, tag="uacc")
                for jt in range(seqt):
                    sim = simp.tile([P, ihw], f32, tag="sim")
                    for c0, cw in chunks(ihw):
                        nc.tensor.matmul(
                            sim[:, c0:c0 + cw],
                            lhsT=kt_sb[hb:hb + hd, mt, jt * P:(jt + 1) * P],
                            rhs=qt_sb[hb:hb + hd, mt, i0 + c0:i0 + c0 + cw],
                            start=True, stop=True)
                    expt = expp.tile([P, ihw], f16, tag="expt")
                    nc.scalar.activation(expt[:], sim[:], Exp, scale=scale)
                    for c0, cw in chunks(ihw):
                        nc.tensor.matmul(
                            uacc[:, c0:c0 + cw],
                            lhsT=v_sb[:, jt, h, :],
                            rhs=expt[:, c0:c0 + cw],
                            start=(jt == 0), stop=(jt == seqt - 1))
                if pre_norm_cb is not None:
                    # emit next phase's projection granules here so their DVE
                    # drains are ordered BEFORE this phase's norm chain
                    pre_norm_cb()
                # normalization: r = 1/s; U_norm = U~ * broadcast(r)
                u_sb = upool.tile([hd, ihw], f16, tag="u")
                nc.vector.tensor_copy(u_sb[:], uacc[0:hd, :])
                if skip_norm:
                    nc.vector.tensor_copy(
                        upairs[pair][hb:hb + hd, i0:i0 + ihw], u_sb[:])
                    return
                srow = rows.tile([65, ihw], f32, tag="srow")
                nc.vector.tensor_copy(srow[64:65, :], uacc[hd:hd + 1, :])
                # r = 1/s. The fused custom-DVE reciprocal op returns garbage
                # on this HW path, so either ACT ln/exp ("ln") or a manual
                # Newton iteration from standard DVE ops ("dve", default —
                # keeps the critical ACT engine free for the softmax exps).
                rrow = rows.tile([65, ihw], f32, tag="rrow")
                rrow16 = rows.tile([65, ihw], f16, tag="rrow16")
                if nmode == "ln":
                    lnrow = rows.tile([65, ihw], f32, tag="lnrow")
                    nc.scalar.activation(lnrow[64:65, :], srow[64:65, :], Ln)
                    nc.scalar.activation(rrow[64:65, :], lnrow[64:65, :], Exp,
                                         scale=-1.0)
                    nc.vector.tensor_copy(rrow16[64:65, :], rrow[64:65, :])
                elif nmode == "dve":
                    i32 = mybir.dt.int32
                    s_r, u_r, t_r = (srow[64:65, :], rrow[64:65, :],
                                     None)
                    trow = rows.tile([65, ihw], f32, tag="trow")
                    t_r = trow[64:65, :]
                    # u0 = bitcast(~bits(s)) * 0.23549792   (u = -1/s approx)
                    nc.vector.tensor_scalar(t_r.bitcast(i32), s_r.bitcast(i32),
                                            -1, None,
                                            op0=mybir.AluOpType.bitwise_xor)
                    nc.vector.tensor_scalar_mul(u_r, t_r, 0.23549792)
                    # two Newton passes: u <- (s*u + c)*u, c = 2.0017324, 2.0
                    for c in (2.0017324, 2.0):
                        nc.vector.tensor_mul(t_r, s_r, u_r)
                        nc.vector.scalar_tensor_tensor(
                            u_r, t_r, float(c), u_r,
                            op0=mybir.AluOpType.add, op1=mybir.AluOpType.mult)
                    # r = -u, cast to fp16
                    nc.vector.tensor_scalar_mul(rrow16[64:65, :], u_r, -1.0)
                elif nmode == "copy":  # timing-only bisect: wrong math
                    nc.vector.tensor_copy(rrow16[64:65, :], srow[64:65, :])
                else:
                    raise ValueError(nmode)
                for c0, cw in chunks(ihw):
                    rps = spare.tile([hd, NC5], f32, tag="ps512")
                    nc.tensor.matmul(
                        rps[:, 0:cw],
                        lhsT=ones65[64:65, :],
                        rhs=rrow16[64:65, c0:c0 + cw],
                        start=True, stop=True)
                    nc.vector.tensor_mul(
                        upairs[pair][hb:hb + hd, i0 + c0:i0 + c0 + cw],
                        u_sb[:, c0:c0 + cw], rps[:, 0:cw])

        def final_proj(ms=None):
            mp = min(P, dmc)
            for m in (range(seqt) if ms is None else ms):
                if y_pair_dma:
                    yt = ysb.tile([P, dout], f32, tag="yt")
                for n0, nw in chunks(dout):
                    yps = spare.tile([P, NC5], f32, tag="ps512")
                    for p in range(npairs):
                        nc.tensor.matmul(
                            yps[:, 0:nw],
                            lhsT=upairs[p][0:mp, m * P:(m + 1) * P],
                            rhs=wo_sb[0:mp, p, n0:n0 + nw],
                            start=(p == 0), stop=(p == npairs - 1))
                    if y_pair_dma:
                        nc.vector.tensor_copy(yt[:, n0:n0 + nw], yps[:, 0:nw])
                    else:
                        yt = ysb.tile([P, NC5], f32, tag="yt")
                        nc.vector.tensor_copy(yt[:, 0:nw], yps[:, 0:nw])
                        nc.sync.dma_start(
                            y[m * P:(m + 1) * P, n0:n0 + nw], yt[:, 0:nw])
                if y_pair_dma:
                    nc.sync.dma_start(y[m * P:(m + 1) * P, :], yt[:])

        # Emission schedule: per-engine instruction order is static after
        # scheduling, so projection granules are threaded between attention
        # (h, ih) phases — each phase's inputs emitted one phase ahead; the
        # ACT-paced attention then hides the remaining projection PE work.
        attn_phases = [(h, ih) for h in range(nh) for ih in range(n_ih)]

        def phase_needs(idx):
            # granules that must be emitted before attention phase idx;
            # every phase's j-loop consumes ALL v tiles, so v has deadline 0.
            if idx >= len(attn_phases):
                return []
            h, ih = attn_phases[idx]
            mt = h // heads_per_mtile
            need = [("k", mt, n0, nw) for n0, nw in chunks(seq)]
            need += [("q", mt, n0, nw) for n0, nw in chunks(seq)
                     if n0 < (ih + 1) * ihw and n0 + nw > ih * ihw]
            if idx == 0:
                need += [("v", jt) for jt in range(seqt)]
            return need

        emitted = set()

        def emit_granules(needs):
            for g in needs:
                if g in emitted:
                    continue
                emitted.add(g)
                if g[0] == "v":
                    proj_v_tile(g[1])
                else:
                    which = 0 if g[0] == "k" else 1
                    proj_kq_tile(g[1], which, g[2], g[3])

        all_granules = []
        for idx in range(len(attn_phases)):
            for g in phase_needs(idx):
                if g not in all_granules:
                    all_granules.append(g)

        if phases == 'proj':
            emit_granules(all_granules)
        else:
            emit_granules(phase_needs(0))
            # deadline-ordered backlog, spread evenly across early boundaries
            backlog = [g for g in all_granules if g not in emitted]
            nb = max(1, len(attn_phases) - 2)
            share = -(-len(backlog) // nb)
            last = len(attn_phases) - 1
            for idx, (h, ih) in enumerate(attn_phases):
                def _cb(idx=idx):
                    emit_granules(phase_needs(idx + 1))
                    take = [g for g in backlog if g not in emitted][:share]
                    emit_granules(take)
                attn_head_ih(h, ih, nm_override="ln" if idx == last else None,
                             pre_norm_cb=_cb)
                if phases == 'all' and idx == last - 1 and n_ih > 1:
                    # final-proj m-tiles whose i-range completes at the
                    # second-to-last phase overlap the last phase's attention
                    lh, lih = attn_phases[last]
                    done_ih = [p_ih for p_ih in range(n_ih) if p_ih != lih]
                    ms = [m for m in range(seqt)
                          if (m * P) // ihw in done_ih]
                    final_proj(ms)
            if phases == 'all':
                lh, lih = attn_phases[last]
                if n_ih > 1:
                    final_proj([m for m in range(seqt)
                                if (m * P) // ihw == lih])
                else:
                    final_proj()

    nc.compile()
    return nc


_NC_CACHE = {}


def _get_nc():
    if "nc" not in _NC_CACHE:
        _NC_CACHE["nc"] = build_nc()
    return _NC_CACHE["nc"]


def _prep_core_inputs(x, Wq, Wkv, Wo):
    """Host-side shard + layout prep: per-core fp16 slices."""
    f16 = np.float16
    in_maps = []
    for c in range(N_CORES):
        b, g = c // 2, c % 2
        s = slice(g * DMC, (g + 1) * DMC)
        in_maps.append({
            "xt": np.ascontiguousarray(x[b].T).astype(f16),
            "wq": np.ascontiguousarray(Wq[:, s]).astype(f16),
            "wk": np.ascontiguousarray(Wkv[:, g * DMC:(g + 1) * DMC]).astype(f16),
            "wv": np.ascontiguousarray(
                Wkv[:, DIM_MODEL + g * DMC:DIM_MODEL + (g + 1) * DMC]).astype(f16),
            "wo": np.ascontiguousarray(Wo[s, :]).astype(f16),
        })
    return in_maps


def kernel(x, Wq, Wkv, Wo, bo):
    from concourse import bass_utils

    x = np.asarray(x, dtype=np.float32)
    Wq = np.asarray(Wq, dtype=np.float32)
    Wkv = np.asarray(Wkv, dtype=np.float32)
    Wo = np.asarray(Wo, dtype=np.float32)
    bo = np.asarray(bo, dtype=np.float32)

    nc = _get_nc()
    in_maps = _prep_core_inputs(x, Wq, Wkv, Wo)
    res = bass_utils.run_bass_kernel_spmd(nc, in_maps,
                                          core_ids=list(range(N_CORES)))
    out = np.empty((B, N, QDIM), dtype=np.float32)
    for b in range(B):
        out[b] = res.results[2 * b]["y"] + res.results[2 * b + 1]["y"] + bo
    return out



# revision 41
# speedup vs baseline: 1.3594x; 1.3594x over previous
"""Multi-head self-attention on 8 Trainium2 NeuronCores.

Problem: x:(4,2048,1024) fp32; q = x@Wq, kv = x@Wkv (k,v split), 8 heads of
dim 64, softmax(q k^T / 8) v, concat heads, @Wo + bo -> (4,2048,1024).

Sharding: core c handles batch b=c//2 and head group g=c%2 (4 of 8 heads).
Each core computes its batch's projections restricted to its 4 heads, full
attention for those heads, and a partial output projection y_c = U_norm @ Wo_g.
Host gathers: out[b] = y_{2b} + y_{2b+1} + bo  (the "all-reduce" of the
tensor-parallel head split, done at unshard time).

Device algorithm (per core), matmul operands fp16, PSUM accumulate fp32:
  - projections: qT/kT (head_dim-major) and v (seq-major, with a ones column
    appended for the softmax denominator), emitted as deadline-scheduled
    granules threaded through the attention phases to fill PE gaps.
  - attention per (i-block of 1024, head): for each j-tile (128 rows):
      simT[j,i] = kT_h(j)^T @ qT_h            (PE, K=64, 2x512 cols)
      expt = exp(SCALE*simT)                  (ACT, PSUM->SBUF fp16)
      U[i128, d|1] += expt_blk^T @ [v_h | 1]  (PE, K=128, 65 cols x 8 blocks)
    U lands i-major, so the softmax denominator s is a per-partition scalar:
    copy U out (frees PSUM fast), r = 1/s via DVE Newton on [128,8], then
    one tensor_scalar per i-subtile: u16 = U * r * (-1 folded).
  - uT for the final projection via DMA crossbar transpose (idle DMA HW):
    ubuf[i, d-pair] -> upair[d-pair, i] in [128,128] tiles.
  - y[m,:] = sum_pairs upair^T @ Wo_pair (PE), DVE drain to fp16, DMA out.
"""

import numpy as np

# ---- problem constants (hardcoded per the harness contract) ----
B, N, QDIM = 4, 2048, 1024
HEADS, DIM_MODEL = 8, 512
HEAD_DIM = DIM_MODEL // HEADS  # 64
SCALE = HEAD_DIM ** -0.5  # 0.125
N_CORES = 8
HEADS_PER_CORE = HEADS // 2  # 4 (head-group split across 2 cores per batch)
DMC = HEADS_PER_CORE * HEAD_DIM  # 256 per-core model dim slice


def build_nc(seq=N, qd=QDIM, nh=HEADS_PER_CORE, hd=HEAD_DIM, dout=QDIM,
             scale=SCALE, ibw=1024, expp_bufs=16, ucp_bufs=2, ysb_bufs=4,
             norm_copy='dve', warmup_mms=12, dve_exp_jts=()):
    """Build the per-core Bass program (same program on all 8 cores)."""
    from contextlib import ExitStack

    import concourse.bass as bass
    import concourse.tile as tile
    from concourse import bacc, mybir

    P = 128
    NC5 = 512  # psum bank width in fp32 / max moving free dim
    f16 = mybir.dt.float16
    f32 = mybir.dt.float32
    i32 = mybir.dt.int32
    i16 = mybir.dt.int16
    Exp = mybir.ActivationFunctionType.Exp
    # fp16 Schraudolph exp: bits16 = A*x + B (float->int16 write), giving
    # exp(scale*x) with ~2% rms error; used to offload the saturated ACT
    # engine for selected j-tiles (the softmax denominator uses the same
    # approximated weights, so the common mode cancels).
    sch_a = float(scale * 1024 * np.log2(np.e))
    sch_b = 15360.0 - 44.0

    dmc = nh * hd                 # per-core projected dim (256)
    kt = qd // P                  # contraction tiles over QDIM (8)
    seqt = seq // P               # seq tiles (16)
    mtiles = dmc // P             # qT/kT partition tiles (2)
    hpm = nh // mtiles            # heads per m-tile / pair (2)
    n_ib = seq // ibw             # i-blocks (2)
    itb = ibw // P                # i-subtiles per block (8)
    npairs = mtiles

    nc = bacc.Bacc("TRN2", target_bir_lowering=False, debug=False,
                   num_devices=N_CORES)

    xt = nc.dram_tensor("xt", (qd, seq), f16, kind="ExternalInput").ap()
    wq = nc.dram_tensor("wq", (qd, dmc), f16, kind="ExternalInput").ap()
    wk = nc.dram_tensor("wk", (qd, dmc), f16, kind="ExternalInput").ap()
    wv = nc.dram_tensor("wv", (qd, dmc), f16, kind="ExternalInput").ap()
    wo = nc.dram_tensor("wo", (dmc, dout), f16, kind="ExternalInput").ap()
    y = nc.dram_tensor("y", (seq, dout), f16, kind="ExternalOutput").ap()

    with tile.TileContext(nc) as tc, ExitStack() as ctx:
        # ---- SBUF pools ----
        persist = ctx.enter_context(tc.tile_pool(name="persist", bufs=1))
        expp = ctx.enter_context(tc.tile_pool(name="expp", bufs=expp_bufs))
        ucp = ctx.enter_context(tc.tile_pool(name="ucp", bufs=ucp_bufs))
        rows = ctx.enter_context(tc.tile_pool(name="rows", bufs=2))
        ysb = ctx.enter_context(tc.tile_pool(name="ysb", bufs=ysb_bufs))
        # ---- PSUM pool (8 banks: 4 sim + 2 uacc + 2 spare) ----
        psum = ctx.enter_context(tc.tile_pool(name="psum", bufs=1,
                                              space="PSUM"))

        # ---- persistent SBUF tensors ----
        xt_sb = persist.tile([P, kt, seq], f16)
        wq_sb = persist.tile([P, kt, dmc], f16)
        wk_sb = persist.tile([P, kt, dmc], f16)
        wv_sb = persist.tile([P, kt, dmc], f16)
        wo_sb = persist.tile([P, npairs, dout], f16)
        v_sb = persist.tile([P, seqt, nh, hd + 1], f16)
        qt_sb = persist.tile([P, mtiles, seq], f16)
        kt_sb = persist.tile([P, mtiles, seq], f16)
        upairs = [persist.tile([P, seq], f16, name=f"upair{p}")
                  for p in range(npairs)]
        ubufs = [persist.tile([P, seqt, P], f16, name=f"ubuf{p}")
                 for p in range(npairs)]

        # ---- input loads; xt split in seq-chunks so the first k/q proj
        # granules can start after ~4.5us instead of waiting the full 4MB ----
        xt_r = xt.rearrange("(ko ki) s -> ki ko s", ki=P)
        nc.sync.dma_start(wk_sb[:], wk.rearrange("(ko ki) m -> ki ko m", ki=P))
        nc.sync.dma_start(xt_sb[:, :, 0:NC5], xt_r[:, :, 0:NC5])
        nc.sync.dma_start(wq_sb[:], wq.rearrange("(ko ki) m -> ki ko m", ki=P))
        nc.sync.dma_start(xt_sb[:, :, NC5:2 * NC5], xt_r[:, :, NC5:2 * NC5])
        nc.sync.dma_start(wv_sb[:], wv.rearrange("(ko ki) m -> ki ko m", ki=P))
        for c in range(2, seq // NC5):
            nc.sync.dma_start(xt_sb[:, :, c * NC5:(c + 1) * NC5],
                              xt_r[:, :, c * NC5:(c + 1) * NC5])
        nc.sync.dma_start(wo_sb[:], wo.rearrange("(t p) n -> p t n", p=P))
        nc.vector.memset(v_sb[:, :, :, hd:hd + 1], 1.0)

        # ---- PE p-state warm-up while the first DMAs land: the Tensor
        # engine takes ~3us of continuous work to reach full clock, so burn
        # the DMA wait on throwaway matmuls and enter the prologue warm.
        if warmup_mms:
            warm = persist.tile([P, NC5], f16)
            nc.vector.memset(warm[:], 0.0)
            for _ in range(warmup_mms):
                wps = psum.tile([P, NC5], f32, tag="ps512", bufs=2, name="wps")
                nc.tensor.matmul(wps[0:hd, :], lhsT=warm[:, 0:hd],
                                 rhs=warm[:], start=True, stop=True)

        # ---- projection / final-proj granules (PE + DVE, interleavable) ----
        def proj_kq_chunk(mt, which, c):
            """[128, 512] chunk of kT (which=0) or qT (which=1), m-tile mt."""
            n0 = c * NC5
            w_sb, out_sb = ((wk_sb, kt_sb), (wq_sb, qt_sb))[which]
            ps = psum.tile([P, NC5], f32, tag="ps512", bufs=2, name="ps")
            for ko in range(kt):
                nc.tensor.matmul(
                    ps[:],
                    lhsT=w_sb[:, ko, mt * P:(mt + 1) * P],
                    rhs=xt_sb[:, ko, n0:n0 + NC5],
                    start=(ko == 0), stop=(ko == kt - 1))
            nc.vector.tensor_copy(out_sb[:, mt, n0:n0 + NC5], ps[:])

        def proj_v_tile(jt):
            """v natural layout [seq, dmc] -> v_sb[:, jt, h, 0:hd]."""
            ps = psum.tile([P, NC5], f32, tag="ps512", bufs=2, name="ps")
            for ko in range(kt):
                nc.tensor.matmul(
                    ps[:, 0:dmc],
                    lhsT=xt_sb[:, ko, jt * P:(jt + 1) * P],
                    rhs=wv_sb[:, ko, :],
                    start=(ko == 0), stop=(ko == kt - 1))
            nc.vector.tensor_copy(
                v_sb[:, jt, :, 0:hd],
                ps[:, 0:dmc].rearrange("p (h d) -> p h d", h=nh))

        tail_rot = [0]

        def final_proj_m(m, tail=False):
            """y[m*128:(m+1)*128, :] = sum_p upair_p[:, m-cols]^T @ Wo_p.
            The two PSUM drains go on different engines so they overlap.
            In the tail the uacc banks are free, so rotate over 4 PSUM
            buffers to keep PE from stalling on drain round-trips."""
            yt = ysb.tile([P, dout], f16, tag="yt")
            for ci, n0 in enumerate(range(0, dout, NC5)):
                if tail and tail_rot[0] % 2:
                    k = tail_rot[0] // 2 % 2
                    ups = psum.tile([P, itb // 2, P], f32, tag=f"uacc{k}",
                                    name="yps")
                    yps = ups.rearrange("p a b -> p (a b)")
                else:
                    yps = psum.tile([P, NC5], f32, tag="ps512", bufs=2,
                                    name="yps")
                tail_rot[0] += 1
                for p in range(npairs):
                    nc.tensor.matmul(
                        yps[:],
                        lhsT=upairs[p][:, m * P:(m + 1) * P],
                        rhs=wo_sb[:, p, n0:n0 + NC5],
                        start=(p == 0), stop=(p == npairs - 1))
                # GPSIMD cannot touch PSUM; in the tail the (idle) ACT
                # engine drains the second chunk instead of DVE.
                if ci == 1 and tail:
                    nc.scalar.copy(yt[:, n0:n0 + NC5], yps[:])
                else:
                    nc.vector.tensor_copy(yt[:, n0:n0 + NC5], yps[:])
            nc.sync.dma_start(y[m * P:(m + 1) * P, :], yt[:])

        emitted = set()

        def emit_granule(g):
            if g in emitted:
                return
            emitted.add(g)
            if g[0] == "v":
                proj_v_tile(g[1])
            elif g[0] == "f":
                final_proj_m(g[1])
            else:
                proj_kq_chunk(g[1], 0 if g[0] == "k" else 1, g[2])

        # ---- granule schedule over global slots (phase, jt) ----
        # Phase order interleaves i-blocks and head-pairs so i-block 0 is
        # fully transposed after phase 5 and its final-proj granules fill
        # the otherwise granule-dry phases 6-7.  Slot s = p*seqt + jt;
        # sched[s] granules are emitted during iteration s-1 (after the
        # lookahead sim, before that iteration's avs).
        phases = [(0, 0), (0, 1), (1, 0), (1, 1),
                  (0, 2), (0, 3), (1, 2), (1, 3)]  # (ib, h)

        def p_first_for(pred):
            return min(p for p, (ib, h) in enumerate(phases) if pred(ib, h))

        sched = {}          # iter s -> [granules]; [0] lands before avs(s)
        prologue_set = {("k", 0, 0), ("q", 0, 0), ("q", 0, 1)}

        def put(slot, g, due=None):
            assert due is None or slot <= due, (slot, due, g)
            if g not in prologue_set:
                sched.setdefault(max(0, slot), []).append(g)

        # q chunks: sim(p_first*seqt) is emitted one iter early, so chunks
        # land >= 2 iters before the phase (phase 0's are in the prologue;
        # the last pair covers the uacc WAR windows at phase starts 64/80)
        for ib in range(n_ib):
            for mt in range(mtiles):
                due = p_first_for(lambda i, h: i == ib and h // hpm == mt) \
                    * seqt - 2
                for ci in range(ibw // NC5):
                    slot = due - 1 + ci if due < 90 else 64 + 16 * ci
                    put(slot, ("q", mt, ib * (ibw // NC5) + ci), due=due)
        # k chunks: sim(p_first(mt), jt) reads kT chunk jt//4.  mt1's go as
        # late as their deadlines allow: the early phases are PE-bound and
        # any extra work there delays every subsequent exp.
        for mt in range(mtiles):
            p_f = p_first_for(lambda i, h: h // hpm == mt)
            for c in range(seq // NC5):
                due = p_f * seqt + 4 * c - 2
                put(due - 2 if mt > 0 else due, ("k", mt, c), due=due)
        # v tiles: phase 0's av(jt) consumes v(jt); just-in-time with a
        # ~2 iteration margin (the expt pool absorbs the av lag).
        for jt in range(seqt):
            put(jt - 2, ("v", jt), due=jt)
        # final-proj granules for i-block 0 fill phases 6-7 (and their
        # phase-start iters 96/112 cover the uacc WAR windows there)
        p_avail = 1 + max(p for p, (ib, h) in enumerate(phases) if ib == 0)
        for idx in range(itb):
            put(p_avail * seqt + 4 * idx, ("f", idx))
        # tail f granules (i-block 1) are emitted after the last phase.

        # ---- attention phases ----
        def head_geom(h):
            mt = h // hpm
            hb = (h % hpm) * hd
            return mt, hb, mt  # pair == mt

        cur = {}  # current phase state: uaccs

        def emit_sim(p, jt, mid_cb=None):
            ib, h = phases[p]
            mt, hb, pair = head_geom(h)
            i0 = ib * ibw
            sim = psum.tile([P, ibw], f32, tag="sim", bufs=2, name="sim")
            for ci, c0 in enumerate(range(0, ibw, NC5)):
                nc.tensor.matmul(
                    sim[:, c0:c0 + NC5],
                    lhsT=kt_sb[hb:hb + hd, mt, jt * P:(jt + 1) * P],
                    rhs=qt_sb[hb:hb + hd, mt, i0 + c0:i0 + c0 + NC5],
                    start=True, stop=True)
                if ci == 0 and mid_cb is not None:
                    mid_cb()
            return sim

        def emit_phase_body(p, sim_jt0):
            """Emit exp/av for all jt of phase p; sims pipelined one ahead."""
            ib, h = phases[p]
            mt, hb, pair = head_geom(h)
            uaccs = [psum.tile([P, itb // 2, P], f32, tag=f"uacc{k}",
                               name=f"uacc{k}") for k in range(2)]
            sim = sim_jt0
            for jt in range(seqt):
                s = p * seqt + jt
                expt = expp.tile([P, ibw], f16, tag="expt")
                if p > 0 and jt in dve_exp_jts:
                    nc.vector.tensor_scalar(
                        expt.bitcast(i16), sim[:], sch_a, sch_b,
                        op0=mybir.AluOpType.mult, op1=mybir.AluOpType.add)
                else:
                    nc.scalar.activation(expt[:], sim[:], Exp, scale=scale)
                # next sim (this phase or first of next phase) keeps PE warm
                # while ACT runs exp; then granules; avs (which wait on exp)
                # come last so PE never head-of-line blocks on ACT.
                if jt < seqt - 1:
                    sim = emit_sim(p, jt + 1)
                elif p + 1 < len(phases):
                    cur["next_sim"] = emit_sim(p + 1, 0)
                gs = sched.get(s, ())
                if gs:
                    emit_granule(gs[0])
                # start=True zeroes the whole 2KB PSUM bank, so only the
                # first sub-accumulator of each bank starts the group (its
                # zero covers the others) and only the last one stops it.
                half = itb // 2
                for g in range(itb):
                    ua = uaccs[g // half]
                    nc.tensor.matmul(
                        ua[:, g % half, 0:hd + 1],
                        lhsT=expt[:, g * P:(g + 1) * P],
                        rhs=v_sb[:, jt, h, :],
                        start=(jt == 0 and g % half == 0),
                        stop=(jt == seqt - 1 and g % half == half - 1))
                for g in gs[1:]:
                    emit_granule(g)
            # ---- normalization: copy U out fast (frees the uacc banks for
            # the next phase), then r=1/s per partition.  The copies go on
            # the otherwise-idle GpSimd engine to shorten that window.  The
            # last phase has no successor, so it skips the staging copy and
            # normalizes straight out of PSUM (shorter tail).
            last = p == len(phases) - 1
            if last:
                sg = rows.tile([P, itb, 1], f32, tag="sgath")
                for k in range(2):
                    nc.vector.tensor_copy(
                        sg[:, (itb // 2) * k:(itb // 2) * (k + 1), :],
                        uaccs[k][:, :, hd:hd + 1])
                s_ap = sg[:]

                def usrc(g):
                    return uaccs[g // (itb // 2)][:, g % (itb // 2), 0:hd]
            else:
                cp = nc.gpsimd if norm_copy == 'pool' else nc.vector
                assert norm_copy == 'dve', "gpsimd cannot access PSUM"
                uc = ucp.tile([P, itb, hd + 1], f32, tag="ucopy")
                for k in range(2):
                    cp.tensor_copy(
                        uc[:, (itb // 2) * k:(itb // 2) * (k + 1), :],
                        uaccs[k][:, :, 0:hd + 1])
                s_ap = uc[:, :, hd:hd + 1]

                def usrc(g):
                    return uc[:, g, 0:hd]
            rr = rows.tile([P, itb, 1], f32, tag="rrow")
            tr = rows.tile([P, itb, 1], f32, tag="trow")
            # u0 = bitcast(~bits(s)) * 0.23549792 approximates -1/s; two
            # Newton passes u <- (s*u + c)*u refine it (c = 2.0017324, 2.0).
            nc.vector.tensor_scalar(tr.bitcast(i32), s_ap.bitcast(i32),
                                    -1, None, op0=mybir.AluOpType.bitwise_xor)
            nc.vector.tensor_scalar_mul(rr, tr, 0.23549792)
            for c in (2.0017324, 2.0):
                nc.vector.tensor_mul(tr, s_ap, rr)
                nc.vector.scalar_tensor_tensor(
                    rr, tr, float(c), rr,
                    op0=mybir.AluOpType.add, op1=mybir.AluOpType.mult)
            # u16 = U * r * (-1)  (fold the -1/s sign into op1)
            for g in range(itb):
                it = ib * itb + g
                nc.vector.tensor_scalar(
                    ubufs[pair][:, it, hb:hb + hd], usrc(g),
                    rr[:, g, :], -1.0,
                    op0=mybir.AluOpType.mult, op1=mybir.AluOpType.mult)
            # pair complete -> transpose to d-major via DMA crossbar
            # (batched: HWDGE overhead is per-instr; the last phase splits
            # in two so the first tail final-proj granules start earlier)
            if h % hpm == hpm - 1:
                nb = 2 if last else 1
                w = itb // nb
                for b in range(nb):
                    t0 = ib * itb + b * w
                    nc.sync.dma_start_transpose(
                        upairs[pair][:, t0 * P:(t0 + w) * P]
                        .rearrange("d (t i) -> d t i", i=P),
                        ubufs[pair][:, t0:t0 + w, :])

        # prologue: k chunk first (its DMA inputs land earliest), then the
        # two q chunks interleaved with sim(0)'s two halves so exp(0)
        # starts as early as possible.
        emit_granule(("k", 0, 0))
        emit_granule(("q", 0, 0))
        first_sim = emit_sim(0, 0,
                             mid_cb=lambda: emit_granule(("q", 0, 1)))
        for p in range(len(phases)):
            emit_phase_body(p, first_sim)
            first_sim = cur.pop("next_sim", None)
        # tail: final projection for i-block 1
        for m in range(itb, seqt):
            final_proj_m(m, tail=True)

    nc.compile()
    return nc


_NC_CACHE = {}


def _get_nc():
    if "nc" not in _NC_CACHE:
        _NC_CACHE["nc"] = build_nc()
    return _NC_CACHE["nc"]


def _prep_core_inputs(x, Wq, Wkv, Wo):
    """Host-side shard + layout prep: per-core fp16 slices."""
    f16 = np.float16
    in_maps = []
    for c in range(N_CORES):
        b, g = c // 2, c % 2
        s = slice(g * DMC, (g + 1) * DMC)
        in_maps.append({
            "xt": np.ascontiguousarray(x[b].T).astype(f16),
            "wq": np.ascontiguousarray(Wq[:, s]).astype(f16),
            "wk": np.ascontiguousarray(Wkv[:, g * DMC:(g + 1) * DMC]).astype(f16),
            "wv": np.ascontiguousarray(
                Wkv[:, DIM_MODEL + g * DMC:DIM_MODEL + (g + 1) * DMC]).astype(f16),
            "wo": np.ascontiguousarray(Wo[s, :]).astype(f16),
        })
    return in_maps


def kernel(x, Wq, Wkv, Wo, bo):
    from concourse import bass_utils

    x = np.asarray(x, dtype=np.float32)
    Wq = np.asarray(Wq, dtype=np.float32)
    Wkv = np.asarray(Wkv, dtype=np.float32)
    Wo = np.asarray(Wo, dtype=np.float32)
    bo = np.asarray(bo, dtype=np.float32)

    nc = _get_nc()
    in_maps = _prep_core_inputs(x, Wq, Wkv, Wo)
    res = bass_utils.run_bass_kernel_spmd(nc, in_maps,
                                          core_ids=list(range(N_CORES)))
    out = np.empty((B, N, QDIM), dtype=np.float32)
    for b in range(B):
        out[b] = (res.results[2 * b]["y"].astype(np.float32)
                  + res.results[2 * b + 1]["y"].astype(np.float32) + bo)
    return out


# revision 63
# speedup vs baseline: 1.4040x; 1.0328x over previous
"""Multi-head self-attention on 8 Trainium2 NeuronCores.

Problem: x:(4,2048,1024) fp32; q = x@Wq, kv = x@Wkv (k,v split), 8 heads of
dim 64, softmax(q k^T / 8) v, concat heads, @Wo + bo -> (4,2048,1024).

Sharding: core c handles batch b=c//2 and head group g=c%2 (4 of 8 heads).
Each core computes its batch's projections restricted to its 4 heads, full
attention for those heads, and a partial output projection y_c = U_norm @ Wo_g.
Host gathers: out[b] = y_{2b} + y_{2b+1} + bo  (the "all-reduce" of the
tensor-parallel head split, done at unshard time).

Device algorithm (per core), matmul operands fp16, PSUM accumulate fp32:
  - projections: qT/kT (head_dim-major) and v (seq-major, with a ones column
    appended for the softmax denominator), emitted as deadline-scheduled
    granules threaded through the attention phases to fill PE gaps.
  - attention per (i-block of 1024, head): for each j-tile (128 rows):
      simT[j,i] = kT_h(j)^T @ qT_h            (PE, K=64, 2x512 cols)
      expt = exp(SCALE*simT)                  (ACT, PSUM->SBUF fp16)
      U[i128, d|1] += expt_blk^T @ [v_h | 1]  (PE, K=128, 65 cols x 8 blocks)
    U lands i-major, so the softmax denominator s is a per-partition scalar:
    copy U out (frees PSUM fast), r = 1/s via DVE Newton on [128,8], then
    one tensor_scalar per i-subtile: u16 = U * r * (-1 folded).
  - uT for the final projection via DMA crossbar transpose (idle DMA HW):
    ubuf[i, d-pair] -> upair[d-pair, i] in [128,128] tiles.
  - y[m,:] = sum_pairs upair^T @ Wo_pair (PE), DVE drain to fp16, DMA out.
"""

import numpy as np

# ---- problem constants (hardcoded per the harness contract) ----
B, N, QDIM = 4, 2048, 1024
HEADS, DIM_MODEL = 8, 512
HEAD_DIM = DIM_MODEL // HEADS  # 64
SCALE = HEAD_DIM ** -0.5  # 0.125
N_CORES = 8
HEADS_PER_CORE = HEADS // 2  # 4 (head-group split across 2 cores per batch)
DMC = HEADS_PER_CORE * HEAD_DIM  # 256 per-core model dim slice


def build_nc(seq=N, qd=QDIM, nh=HEADS_PER_CORE, hd=HEAD_DIM, dout=QDIM,
             scale=SCALE, ibw=1024, expp_bufs=16, ucp_bufs=2, ysb_bufs=6,
             norm_copy='dve', warmup_mms=12, exp_offloads=3):
    """Build the per-core Bass program (same program on all 8 cores)."""
    from contextlib import ExitStack

    import concourse.bass as bass
    import concourse.tile as tile
    from concourse import bacc, mybir

    P = 128
    NC5 = 512  # psum bank width in fp32 / max moving free dim
    f16 = mybir.dt.float16
    f32 = mybir.dt.float32
    i32 = mybir.dt.int32
    i16 = mybir.dt.int16
    Exp = mybir.ActivationFunctionType.Exp
    Ln = mybir.ActivationFunctionType.Ln
    # fp16 Schraudolph exp: bits16 = A*x + B (float->int16 write), giving
    # exp(scale*x) with ~2% rms error; used to offload the saturated ACT
    # engine for selected j-tiles (the softmax denominator uses the same
    # approximated weights, so the common mode cancels).
    sch_a = float(scale * 1024 * np.log2(np.e))
    sch_b = 15360.0 - 44.0

    dmc = nh * hd                 # per-core projected dim (256)
    kt = qd // P                  # contraction tiles over QDIM (8)
    seqt = seq // P               # seq tiles (16)
    mtiles = dmc // P             # qT/kT partition tiles (2)
    hpm = nh // mtiles            # heads per m-tile / pair (2)
    n_ib = seq // ibw             # i-blocks (2)
    itb = ibw // P                # i-subtiles per block (8)
    npairs = mtiles

    nc = bacc.Bacc("TRN2", target_bir_lowering=False, debug=False,
                   num_devices=N_CORES)

    xt = nc.dram_tensor("xt", (qd, seq), f16, kind="ExternalInput").ap()
    wq = nc.dram_tensor("wq", (qd, dmc), f16, kind="ExternalInput").ap()
    wk = nc.dram_tensor("wk", (qd, dmc), f16, kind="ExternalInput").ap()
    wv = nc.dram_tensor("wv", (qd, dmc), f16, kind="ExternalInput").ap()
    wo = nc.dram_tensor("wo", (dmc, dout), f16, kind="ExternalInput").ap()
    y = nc.dram_tensor("y", (seq, dout), f16, kind="ExternalOutput").ap()

    with tile.TileContext(nc) as tc, ExitStack() as ctx:
        # ---- SBUF pools ----
        persist = ctx.enter_context(tc.tile_pool(name="persist", bufs=1))
        expp = ctx.enter_context(tc.tile_pool(name="expp", bufs=expp_bufs))
        ucp = ctx.enter_context(tc.tile_pool(name="ucp", bufs=ucp_bufs))
        rows = ctx.enter_context(tc.tile_pool(name="rows", bufs=2))
        ysb = ctx.enter_context(tc.tile_pool(name="ysb", bufs=ysb_bufs))
        # ---- PSUM pool (8 banks: 4 sim + 2 uacc + 2 spare) ----
        psum = ctx.enter_context(tc.tile_pool(name="psum", bufs=1,
                                              space="PSUM"))

        # ---- persistent SBUF tensors ----
        xt_sb = persist.tile([P, kt, seq], f16)
        wq_sb = persist.tile([P, kt, dmc], f16)
        wk_sb = persist.tile([P, kt, dmc], f16)
        wv_sb = persist.tile([P, kt, dmc], f16)
        wo_sb = persist.tile([P, npairs, dout], f16)
        v_sb = persist.tile([P, seqt, nh, hd + 1], f16)
        qt_sb = persist.tile([P, mtiles, seq], f16)
        kt_sb = persist.tile([P, mtiles, seq], f16)
        upairs = [persist.tile([P, seq], f16, name=f"upair{p}")
                  for p in range(npairs)]
        ubufs = [persist.tile([P, seqt, P], f16, name=f"ubuf{p}")
                 for p in range(npairs)]
        ident = persist.tile([P, P], f16)

        # ---- input loads; xt split in seq-chunks so the first k/q proj
        # granules can start after ~4.5us instead of waiting the full 4MB ----
        xt_r = xt.rearrange("(ko ki) s -> ki ko s", ki=P)

        def xt_load(c0, c1):
            nc.sync.dma_start(xt_sb[:, :, c0:c1], xt_r[:, :, c0:c1])

        nc.sync.dma_start(wk_sb[:], wk.rearrange("(ko ki) m -> ki ko m", ki=P))
        xt_load(0, 256)
        xt_load(256, NC5)
        nc.sync.dma_start(wq_sb[:], wq.rearrange("(ko ki) m -> ki ko m", ki=P))
        xt_load(NC5, 2 * NC5)
        nc.sync.dma_start(wv_sb[:], wv.rearrange("(ko ki) m -> ki ko m", ki=P))
        for c in range(2, seq // NC5):
            xt_load(c * NC5, (c + 1) * NC5)
        nc.sync.dma_start(wo_sb[:], wo.rearrange("(t p) n -> p t n", p=P))
        nc.vector.memset(v_sb[:, :, :, hd:hd + 1], 1.0)
        from concourse.masks import make_identity
        make_identity(nc, ident[:])

        # ---- PE p-state warm-up while the first DMAs land: the Tensor
        # engine takes ~3us of continuous work to reach full clock, so burn
        # the DMA wait on throwaway matmuls and enter the prologue warm.
        if warmup_mms:
            warm = persist.tile([P, NC5], f16)
            nc.vector.memset(warm[:], 0.0)
            for _ in range(warmup_mms):
                wps = psum.tile([P, NC5], f32, tag="ps512", bufs=2, name="wps")
                nc.tensor.matmul(wps[0:hd, :], lhsT=warm[:, 0:hd],
                                 rhs=warm[:], start=True, stop=True)

        # ---- projection / final-proj granules (PE + DVE, interleavable) ----
        def proj_kq_chunk(mt, which, c):
            """[128, 512] chunk of kT (which=0) or qT (which=1), m-tile mt."""
            n0 = c * NC5
            w_sb, out_sb = ((wk_sb, kt_sb), (wq_sb, qt_sb))[which]
            ps = psum.tile([P, NC5], f32, tag="ps512", bufs=2, name="ps")
            for ko in range(kt):
                nc.tensor.matmul(
                    ps[:],
                    lhsT=w_sb[:, ko, mt * P:(mt + 1) * P],
                    rhs=xt_sb[:, ko, n0:n0 + NC5],
                    start=(ko == 0), stop=(ko == kt - 1))
            nc.vector.tensor_copy(out_sb[:, mt, n0:n0 + NC5], ps[:])

        def proj_v_tile(jt):
            """v natural layout [seq, dmc] -> v_sb[:, jt, h, 0:hd]."""
            ps = psum.tile([P, NC5], f32, tag="ps512", bufs=2, name="ps")
            for ko in range(kt):
                nc.tensor.matmul(
                    ps[:, 0:dmc],
                    lhsT=xt_sb[:, ko, jt * P:(jt + 1) * P],
                    rhs=wv_sb[:, ko, :],
                    start=(ko == 0), stop=(ko == kt - 1))
            nc.vector.tensor_copy(
                v_sb[:, jt, :, 0:hd],
                ps[:, 0:dmc].rearrange("p (h d) -> p h d", h=nh))

        tail_rot = [0]

        def final_proj_m(m, tail=False):
            """y[m*128:(m+1)*128, :] = sum_p upair_p[:, m-cols]^T @ Wo_p.
            The two PSUM drains go on different engines so they overlap.
            In the tail the uacc banks are free, so rotate over 4 PSUM
            buffers to keep PE from stalling on drain round-trips."""
            yt = ysb.tile([P, dout], f16, tag="yt")
            for ci, n0 in enumerate(range(0, dout, NC5)):
                if tail and tail_rot[0] % 2:
                    k = tail_rot[0] // 2 % 2
                    ups = psum.tile([P, itb // 2, P], f32, tag=f"uacc{k}",
                                    name="yps")
                    yps = ups.rearrange("p a b -> p (a b)")
                else:
                    yps = psum.tile([P, NC5], f32, tag="ps512", bufs=2,
                                    name="yps")
                tail_rot[0] += 1
                for p in range(npairs):
                    nc.tensor.matmul(
                        yps[:],
                        lhsT=upairs[p][:, m * P:(m + 1) * P],
                        rhs=wo_sb[:, p, n0:n0 + NC5],
                        start=(p == 0), stop=(p == npairs - 1))
                # GPSIMD cannot touch PSUM; in the tail the (idle) ACT
                # engine drains the second chunk instead of DVE.
                if ci == 1 and tail:
                    nc.scalar.copy(yt[:, n0:n0 + NC5], yps[:])
                else:
                    nc.vector.tensor_copy(yt[:, n0:n0 + NC5], yps[:])
            nc.sync.dma_start(y[m * P:(m + 1) * P, :], yt[:])

        emitted = set()

        def emit_granule(g):
            if g in emitted:
                return
            emitted.add(g)
            if g[0] == "v":
                proj_v_tile(g[1])
            elif g[0] == "f":
                final_proj_m(g[1])
            else:
                proj_kq_chunk(g[1], 0 if g[0] == "k" else 1, g[2])

        # ---- granule schedule over global slots (phase, jt) ----
        # Phase order interleaves i-blocks and head-pairs so i-block 0 is
        # fully transposed after phase 5 and its final-proj granules fill
        # the otherwise granule-dry phases 6-7.  Slot s = p*seqt + jt;
        # sched[s] granules are emitted during iteration s-1 (after the
        # lookahead sim, before that iteration's avs).
        phases = [(0, 0), (0, 1), (1, 0), (1, 1),
                  (0, 2), (0, 3), (1, 2), (1, 3)]  # (ib, h)

        def p_first_for(pred):
            return min(p for p, (ib, h) in enumerate(phases) if pred(ib, h))

        sched = {}          # iter s -> [granules]; [0] lands before avs(s)
        prologue_set = {("k", 0, 0), ("q", 0, 0), ("q", 0, 1)}

        def put(slot, g, due=None):
            assert due is None or slot <= due, (slot, due, g)
            if g not in prologue_set:
                sched.setdefault(max(0, slot), []).append(g)

        # q chunks: sim(p_first*seqt) is emitted one iter early, so chunks
        # land >= 2 iters before the phase (phase 0's are in the prologue;
        # the last pair covers the uacc WAR windows at phase starts 64/80)
        for ib in range(n_ib):
            for mt in range(mtiles):
                due = p_first_for(lambda i, h: i == ib and h // hpm == mt) \
                    * seqt - 2
                for ci in range(ibw // NC5):
                    slot = due - 1 + ci if due < 90 else 64 + 16 * ci
                    put(slot, ("q", mt, ib * (ibw // NC5) + ci), due=due)
        # k chunks: sim(p_first(mt), jt) reads kT chunk jt//4.  mt1's go as
        # late as their deadlines allow: the early phases are PE-bound and
        # any extra work there delays every subsequent exp.
        for mt in range(mtiles):
            p_f = p_first_for(lambda i, h: h // hpm == mt)
            for c in range(seq // NC5):
                due = p_f * seqt + 4 * c - 2
                put(due - 2 if mt > 0 else due, ("k", mt, c), due=due)
        # v tiles: phase 0's av(jt) consumes v(jt); just-in-time with a
        # ~2 iteration margin (the expt pool absorbs the av lag).
        for jt in range(seqt):
            put(jt - 2, ("v", jt), due=jt)
        # final-proj granules for i-block 0 fill phases 6-7 (and their
        # phase-start iters 96/112 cover the uacc WAR windows there)
        p_avail = 1 + max(p for p, (ib, h) in enumerate(phases) if ib == 0)
        for idx in range(itb):
            put(p_avail * seqt + 4 * idx, ("f", idx))
        # tail f granules (i-block 1) are emitted after the last phase.

        # ---- exp offload slots: a few exps per phase go to DVE via the
        # fp16 Schraudolph trick, with their sims staged in the spare ps512
        # PSUM pair.  That leaves the sim ring a full iteration ahead, so
        # ACT genuinely skips the offloaded tile (~1us each).  Only
        # granule-free iteration windows qualify (the stage borrows the
        # granule PSUM slots).
        offl = {}
        for p in range(1, len(phases)):
            picked = []
            for jt in range(4, 15):
                s0 = p * seqt + jt
                if any(sched.get(s0 + d) for d in (-1, 0, 1, 2)):
                    continue
                if picked and jt - picked[-1] < 4:
                    continue
                picked.append(jt)
                if len(picked) >= exp_offloads:
                    break
            offl[p] = set(picked)

        # ---- attention phases ----
        def head_geom(h):
            mt = h // hpm
            hb = (h % hpm) * hd
            return mt, hb, mt  # pair == mt

        cur = {}  # current phase state: uaccs

        def emit_sim(p, jt):
            ib, h = phases[p]
            mt, hb, pair = head_geom(h)
            i0 = ib * ibw
            offload = jt in offl.get(p, ())
            if offload:
                chunks_out = [psum.tile([P, NC5], f32, tag="ps512", bufs=2,
                                        name="simo") for _ in range(2)]
            else:
                sim = psum.tile([P, ibw], f32, tag="sim", bufs=2, name="sim")
                chunks_out = [sim[:, c0:c0 + NC5]
                              for c0 in range(0, ibw, NC5)]
            for ci, c0 in enumerate(range(0, ibw, NC5)):
                nc.tensor.matmul(
                    chunks_out[ci][:, 0:NC5] if offload else chunks_out[ci],
                    lhsT=kt_sb[hb:hb + hd, mt, jt * P:(jt + 1) * P],
                    rhs=qt_sb[hb:hb + hd, mt, i0 + c0:i0 + c0 + NC5],
                    start=True, stop=True)
            return ("off", chunks_out) if offload else sim

        def emit_phase_body(p, sim_jt0):
            """Emit exp/av for all jt of phase p; sims pipelined one ahead."""
            ib, h = phases[p]
            mt, hb, pair = head_geom(h)
            uaccs = [psum.tile([P, itb // 2, P], f32, tag=f"uacc{k}",
                               name=f"uacc{k}") for k in range(2)]
            sim_handles = {0: sim_jt0}
            for jt in range(seqt):
                s = p * seqt + jt
                sim = sim_handles.pop(jt)
                expt = expp.tile([P, ibw], f16, tag="expt")
                if isinstance(sim, tuple):
                    ei = expt.bitcast(i16)
                    for ci, c0 in enumerate(range(0, ibw, NC5)):
                        nc.vector.tensor_scalar(
                            ei[:, c0:c0 + NC5], sim[1][ci][:], sch_a, sch_b,
                            op0=mybir.AluOpType.mult, op1=mybir.AluOpType.add)
                else:
                    nc.scalar.activation(expt[:], sim[:], Exp, scale=scale)
                # next sim (this phase or first of next phase) keeps PE warm
                # while ACT runs exp; then granules; avs (which wait on exp)
                # come last so PE never head-of-line blocks on ACT.  When
                # jt+1 is an offloaded tile, also emit sim(jt+2) now: its
                # ring slot is already free and pre-staging it is what lets
                # ACT skip the offloaded tile outright.
                targets = [jt + 1]
                if jt + 1 in offl.get(p, ()):
                    targets.append(jt + 2)
                for t in targets:
                    if t < seqt and t not in sim_handles:
                        sim_handles[t] = emit_sim(p, t)
                if jt == seqt - 1 and p + 1 < len(phases):
                    cur["next_sim"] = emit_sim(p + 1, 0)
                gs = sched.get(s, ())
                if gs:
                    emit_granule(gs[0])
                # start=True zeroes the whole 2KB PSUM bank, so only the
                # first sub-accumulator of each bank starts the group (its
                # zero covers the others) and only the last one stops it.
                half = itb // 2
                for g in range(itb):
                    ua = uaccs[g // half]
                    nc.tensor.matmul(
                        ua[:, g % half, 0:hd + 1],
                        lhsT=expt[:, g * P:(g + 1) * P],
                        rhs=v_sb[:, jt, h, :],
                        start=(jt == 0 and g % half == 0),
                        stop=(jt == seqt - 1 and g % half == half - 1))
                for g in gs[1:]:
                    emit_granule(g)
            # ---- normalization: copy U out fast (frees the uacc banks for
            # the next phase), then r=1/s per partition.  The copies go on
            # the otherwise-idle GpSimd engine to shorten that window.  The
            # last phase has no successor, so it skips the staging copy and
            # normalizes straight out of PSUM (shorter tail).
            last = p == len(phases) - 1
            if last:
                # tail: gather s straight from PSUM, Newton r = 1/s on DVE
                # (Ln/Exp on ACT would be shorter but costs two 1.3us
                # activation-table swaps).
                half = itb // 2
                sg = rows.tile([P, itb, 1], f32, tag="sgath")
                for k in range(2):
                    nc.vector.tensor_copy(
                        sg[:, half * k:half * (k + 1), :],
                        uaccs[k][:, :, hd:hd + 1])
                rr = rows.tile([P, itb, 1], f32, tag="rrow")
                tr = rows.tile([P, itb, 1], f32, tag="trow")
                nc.vector.tensor_scalar(tr.bitcast(i32), sg.bitcast(i32),
                                        -1, None,
                                        op0=mybir.AluOpType.bitwise_xor)
                nc.vector.tensor_scalar_mul(rr, tr, 0.23549792)
                for c in (2.0017324, 2.0):
                    nc.vector.tensor_mul(tr, sg, rr)
                    nc.vector.scalar_tensor_tensor(
                        rr, tr, float(c), rr,
                        op0=mybir.AluOpType.add, op1=mybir.AluOpType.mult)
                rpos = rows.tile([P, itb, 1], f32, tag="rpos")
                nc.vector.tensor_scalar_mul(rpos, rr, -1.0)

                def usrc(g):
                    return uaccs[g // half][:, g % half, 0:hd]
            else:
                cp = nc.gpsimd if norm_copy == 'pool' else nc.vector
                assert norm_copy == 'dve', "gpsimd cannot access PSUM"
                uc = ucp.tile([P, itb, hd + 1], f32, tag="ucopy")
                for k in range(2):
                    cp.tensor_copy(
                        uc[:, (itb // 2) * k:(itb // 2) * (k + 1), :],
                        uaccs[k][:, :, 0:hd + 1])
                s_ap = uc[:, :, hd:hd + 1]

                def usrc(g):
                    return uc[:, g, 0:hd]
                rr = rows.tile([P, itb, 1], f32, tag="rrow")
                tr = rows.tile([P, itb, 1], f32, tag="trow")
                # u0 = bitcast(~bits(s)) * 0.23549792 approximates -1/s; two
                # Newton passes u <- (s*u + c)*u refine (c = 2.0017324, 2.0).
                nc.vector.tensor_scalar(tr.bitcast(i32), s_ap.bitcast(i32),
                                        -1, None,
                                        op0=mybir.AluOpType.bitwise_xor)
                nc.vector.tensor_scalar_mul(rr, tr, 0.23549792)
                for c in (2.0017324, 2.0):
                    nc.vector.tensor_mul(tr, s_ap, rr)
                    nc.vector.scalar_tensor_tensor(
                        rr, tr, float(c), rr,
                        op0=mybir.AluOpType.add, op1=mybir.AluOpType.mult)
            # u16 = U * r; mid-flight r is negative (-1/s from the Newton
            # bit-trick) so the -1 folds into op1; the tail's ACT-computed r
            # is positive and the muls split across DVE (even g) and ACT.
            for g in range(itb):
                it = ib * itb + g
                if last and g % 2 == 1:
                    nc.scalar.mul(ubufs[pair][:, it, hb:hb + hd], usrc(g),
                                  rpos[:, g, :])
                elif last:
                    nc.vector.tensor_scalar_mul(
                        ubufs[pair][:, it, hb:hb + hd], usrc(g),
                        rpos[:, g, :])
                else:
                    nc.vector.tensor_scalar(
                        ubufs[pair][:, it, hb:hb + hd], usrc(g),
                        rr[:, g, :], -1.0,
                        op0=mybir.AluOpType.mult, op1=mybir.AluOpType.mult)
            # pair complete -> transpose to d-major.  Mid-flight this rides
            # the idle DMA crossbar (one batched instr, zero engine cost);
            # in the tail PE + both copy engines are idle and the PE
            # identity-transpose path avoids the serial HWDGE+sem latency.
            if h % hpm == hpm - 1 and not last:
                t0 = ib * itb
                nc.sync.dma_start_transpose(
                    upairs[pair][:, t0 * P:(t0 + itb) * P]
                    .rearrange("d (t i) -> d t i", i=P),
                    ubufs[pair][:, t0:t0 + itb, :])
            elif h % hpm == hpm - 1:
                for g in range(itb):
                    it = ib * itb + g
                    tps = psum.tile([P, P], f16, tag="ps512", bufs=2,
                                    name="tps")
                    nc.tensor.transpose(tps[:], ubufs[pair][:, it, :],
                                        ident[:])
                    eng = nc.vector if g % 2 == 0 else nc.scalar
                    if g % 2 == 0:
                        nc.vector.tensor_copy(
                            upairs[pair][:, it * P:(it + 1) * P], tps[:])
                    else:
                        nc.scalar.copy(
                            upairs[pair][:, it * P:(it + 1) * P], tps[:])

        # prologue: k chunk first (its DMA inputs land earliest), then the
        # two q chunks; sim(0) last (PSUM tile deps are whole-tile, so
        # nothing is gained by interleaving its chunks with the q proj).
        emit_granule(("k", 0, 0))
        emit_granule(("q", 0, 0))
        emit_granule(("q", 0, 1))
        first_sim = emit_sim(0, 0)
        for p in range(len(phases)):
            emit_phase_body(p, first_sim)
            first_sim = cur.pop("next_sim", None)
        # tail: final projection for i-block 1
        for m in range(itb, seqt):
            final_proj_m(m, tail=True)

    nc.compile()
    return nc


_NC_CACHE = {}


def _get_nc():
    if "nc" not in _NC_CACHE:
        _NC_CACHE["nc"] = build_nc()
    return _NC_CACHE["nc"]


def _prep_core_inputs(x, Wq, Wkv, Wo):
    """Host-side shard + layout prep: per-core fp16 slices."""
    f16 = np.float16
    in_maps = []
    for c in range(N_CORES):
        b, g = c // 2, c % 2
        s = slice(g * DMC, (g + 1) * DMC)
        in_maps.append({
            "xt": np.ascontiguousarray(x[b].T).astype(f16),
            "wq": np.ascontiguousarray(Wq[:, s]).astype(f16),
            "wk": np.ascontiguousarray(Wkv[:, g * DMC:(g + 1) * DMC]).astype(f16),
            "wv": np.ascontiguousarray(
                Wkv[:, DIM_MODEL + g * DMC:DIM_MODEL + (g + 1) * DMC]).astype(f16),
            "wo": np.ascontiguousarray(Wo[s, :]).astype(f16),
        })
    return in_maps


def kernel(x, Wq, Wkv, Wo, bo):
    from concourse import bass_utils

    x = np.asarray(x, dtype=np.float32)
    Wq = np.asarray(Wq, dtype=np.float32)
    Wkv = np.asarray(Wkv, dtype=np.float32)
    Wo = np.asarray(Wo, dtype=np.float32)
    bo = np.asarray(bo, dtype=np.float32)

    nc = _get_nc()
    in_maps = _prep_core_inputs(x, Wq, Wkv, Wo)
    res = bass_utils.run_bass_kernel_spmd(nc, in_maps,
                                          core_ids=list(range(N_CORES)))
    out = np.empty((B, N, QDIM), dtype=np.float32)
    for b in range(B):
        out[b] = (res.results[2 * b]["y"].astype(np.float32)
                  + res.results[2 * b + 1]["y"].astype(np.float32) + bo)
    return out
